# revision 18
# baseline (speedup 1.0000x reference)
"""Trainium2 Bass kernel for masked attention scoring (sparse_attention).

Computes, per batch b:
    proj = y @ M^T                      # [B, D]
    eij  = tanh(einsum('bsd,bd->bs', x, proj))
    a    = exp(eij) * mask
    a    = a / (sum_s a + EPS)

Sharding: data-parallel over batch B=32 across 8 NeuronCores (4 batches
per core) for x/mask/out. The proj GEMM is sharded over the contraction
dim d: core i receives M[:, i*128:(i+1)*128] and y[:, i*128:(i+1)*128]
(pre-sliced host-side so the SPMD graph is identical), computes partial
proj for ALL 32 batches, and a ReduceScatter(add) hands each core the
final proj rows for exactly its own 4 batches. This cuts per-core M
traffic from 4 MiB to 512 KiB, so the x stream owns the HBM bandwidth.

Per-core device algorithm (memory-bound, x-stream dominated):
  - setup: Msl/ysl loaded (one small DMA each), cast to f16 on VectorE,
    transposed on TensorE, partial proj = ysl^T @ Msl^T in PSUM f32,
    ReduceScatter over DRAM bounce buffers, then proj broadcast across
    partitions via selector matmuls on TensorE.
  - main pass: stream x in [128, 4, 1024] f32 tiles (natural layout,
    2 MiB DMAs) and compute the d-reduction with ONE fused DVE op per
    s-chunk: scalar_tensor_tensor(mult, mult, accum_out) -> eij column.
  - epilogue: tanh+exp per batch on ScalarE as batches finish; then mask
    multiply, free-dim reduce, partition reduce + denominator broadcast
    via tiny TensorE matmuls, normalize, PE-transpose, contiguous DMA
    out. No strided elementwise DMAs anywhere.
"""

import os
import sys

import numpy as np

for _p in ("/opt/trn_rl_repo",):
    if os.path.isdir(_p) and _p not in sys.path:
        sys.path.insert(0, _p)

B, S, D = 32, 2048, 1024
NCORES = 8
BL = B // NCORES        # batches per core
P = 128                 # SBUF partitions
J = S // P              # 16 s-chunks per batch
HALF = J // 4           # s-chunks per x DMA (2 MiB)
DC = D // P             # 8 d-chunks
EPS = 1e-7

_CACHE = {}


def _build():
    import concourse.bacc as bacc
    import concourse.tile as tile
    from concourse import mybir
    from concourse.masks import make_identity

    f32 = mybir.dt.float32
    f16 = mybir.dt.float16
    i32 = mybir.dt.int32

    nc = bacc.Bacc("TRN2", target_bir_lowering=False, debug=False,
                   num_devices=NCORES)

    x_ext = nc.dram_tensor("x", [BL, S, D], f32, kind="ExternalInput").ap()
    ysl_ext = nc.dram_tensor("ysl", [B, P], f32, kind="ExternalInput").ap()
    mask_ext = nc.dram_tensor("mask", [BL, S], i32, kind="ExternalInput").ap()
    msl_ext = nc.dram_tensor("Msl", [D, P], f32, kind="ExternalInput").ap()
    out_ext = nc.dram_tensor("out", [BL, S], f32, kind="ExternalOutput").ap()
    rs_in = nc.dram_tensor("rs_in", [B, D], f32).ap()
    rs_out = nc.dram_tensor("rs_out", [BL, D], f32).ap()

    with tile.TileContext(nc) as tc:
        with (
            tc.tile_pool(name="consts", bufs=1) as consts,
            tc.tile_pool(name="psum_t", bufs=2, space="PSUM") as psum_t_pool,
            tc.tile_pool(name="psum_proj", bufs=1, space="PSUM") as psum_proj_pool,
            tc.tile_pool(name="psum_pb", bufs=1, space="PSUM") as psum_pb_pool,
            tc.tile_pool(name="psum_small", bufs=1, space="PSUM") as psum_small_pool,
            tc.tile_pool(name="xpool", bufs=8) as xpool,
            tc.tile_pool(name="scr", bufs=2) as scr_pool,
        ):
            identity16 = consts.tile([P, P], f16)
            make_identity(nc, identity16)
            identity32 = consts.tile([P, P], f32)
            make_identity(nc, identity32)
            ones_col = consts.tile([P, 1], f32)
            nc.vector.memset(ones_col, 1.0)
            ones_row = consts.tile([1, P], f32)
            nc.vector.memset(ones_row, 1.0)
            eps_t = consts.tile([1, 1], f32)
            nc.vector.memset(eps_t, EPS)

            # ---- M^T slice in f16: one 512KB load, cast, transpose ----
            # mTsl[p_dd, ec, e'] = M[ec*128+e', core_d0 + p_dd]
            msl = consts.tile([P, DC, P], f32)
            nc.sync.dma_start(
                out=msl,
                in_=msl_ext.rearrange("(ec p) dd -> p ec dd", p=P))
            msl16 = consts.tile([P, DC, P], f16)
            nc.vector.tensor_copy(msl16, msl)
            pt = psum_t_pool.tile([P, DC, P], f16, tag="pt")
            for ec in range(DC):
                nc.tensor.transpose(pt[:, ec, :], msl16[:, ec, :],
                                    identity16)
            mTsl = consts.tile([P, DC, P], f16)
            nc.vector.tensor_copy(mTsl, pt)

            # ---- y slice -> yTsl[p_dd, b] in f16 ----
            ysl = consts.tile([B, P], f32)
            nc.sync.dma_start(out=ysl, in_=ysl_ext)
            ysl16 = consts.tile([B, P], f16)
            nc.vector.tensor_copy(ysl16, ysl)
            yt_ps = psum_small_pool.tile([P, B], f16, tag="small")
            nc.tensor.transpose(yt_ps, ysl16, identity16[:B, :B])
            yTsl = consts.tile([P, B], f16)
            nc.vector.tensor_copy(yTsl, yt_ps)

            # ---- partial proj for ALL batches + ReduceScatter ----
            pproj_ps = psum_proj_pool.tile([B, D], f32)
            for eh in range(2):
                nc.tensor.matmul(
                    pproj_ps[:, eh * 512:(eh + 1) * 512],
                    lhsT=yTsl,
                    rhs=mTsl[:, eh * 4:(eh + 1) * 4, :],
                    start=True, stop=True)
            pproj_sb = consts.tile([B, D], f32)
            nc.vector.tensor_copy(pproj_sb, pproj_ps)
            nc.sync.dma_start(out=rs_in, in_=pproj_sb)
            nc.gpsimd.collective_compute(
                "ReduceScatter",
                mybir.AluOpType.add,
                replica_groups=[list(range(NCORES))],
                ins=[rs_in],
                outs=[rs_out],
            )
            # own 4 batches' proj rows, cast to f16 during DMA back
            proj_sb = consts.tile([BL, D], f16)
            nc.gpsimd.dma_start(out=proj_sb, in_=rs_out)

            # ---- broadcast proj rows across partitions via TensorE ----
            projbc = []
            for b in range(BL):
                sel = consts.tile([BL, P], f16, name=f"sel{b}")
                nc.gpsimd.memset(sel, 0.0)
                nc.gpsimd.affine_select(
                    out=sel, in_=sel,
                    compare_op=mybir.AluOpType.not_equal,
                    fill=1.0, base=-b,
                    pattern=[[0, P]], channel_multiplier=1)
                pb = consts.tile([P, D], f32, name=f"projbc{b}")
                for eh in range(2):
                    pb_ps = psum_pb_pool.tile([P, 512], f32, tag="pbps")
                    nc.tensor.matmul(
                        pb_ps,
                        lhsT=sel,
                        rhs=proj_sb[:, eh * 512:(eh + 1) * 512],
                        start=True, stop=True)
                    if b == 0:
                        nc.vector.tensor_copy(
                            pb[:, eh * 512:(eh + 1) * 512], pb_ps)
                    else:
                        nc.scalar.copy(pb[:, eh * 512:(eh + 1) * 512], pb_ps)
                projbc.append(pb)

            # ---- masks: one contiguous cast-DMA + PE transposes ----
            mk_nat = consts.tile([J, BL, P], f32)
            nc.gpsimd.dma_start(
                out=mk_nat,
                in_=mask_ext.rearrange("b (j p) -> j b p", p=P))
            mask_all = consts.tile([P, BL, J], f32)
            for b in range(BL):
                mk_ps = psum_small_pool.tile([P, J], f32, tag="small")
                nc.tensor.transpose(mk_ps, mk_nat[:, b, :], identity32[:J, :J])
                nc.scalar.copy(mask_all[:, b, :], mk_ps)

            # ---- main pass: eij[p, b, col] = x[b, s, :] . proj[b, :] ----
            eij = consts.tile([P, BL, J], f32)
            th = consts.tile([P, BL, J], f32)
            ex = consts.tile([P, BL, J], f32)
            for b in range(BL):
                for half in range(4):
                    xt = xpool.tile([P, HALF, D], f32, tag="xt")
                    nc.sync.dma_start(
                        out=xt,
                        in_=x_ext[b, half * HALF * P:(half + 1) * HALF * P, :]
                        .rearrange("(j p) d -> p j d", p=P),
                    )
                    for j in range(HALF):
                        col = half * HALF + j
                        scr = scr_pool.tile([P, D], f32, tag="scr")
                        nc.vector.scalar_tensor_tensor(
                            out=scr,
                            in0=xt[:, j, :],
                            scalar=1.0,
                            in1=projbc[b],
                            op0=mybir.AluOpType.mult,
                            op1=mybir.AluOpType.mult,
                            accum_out=eij[:, b, col:col + 1],
                        )
                nc.scalar.activation(th[:, b, :], eij[:, b, :],
                                     mybir.ActivationFunctionType.Tanh)
                nc.scalar.activation(ex[:, b, :], th[:, b, :],
                                     mybir.ActivationFunctionType.Exp)

            # ---- fused epilogue over all batches ----
            au = consts.tile([P, BL, J], f32)
            nc.vector.tensor_mul(au, ex, mask_all)
            cs = consts.tile([P, BL], f32)
            nc.vector.reduce_sum(cs, au, axis=mybir.AxisListType.X)
            tot_ps = psum_small_pool.tile([1, BL], f32, tag="small")
            nc.tensor.matmul(tot_ps, lhsT=ones_col, rhs=cs,
                             start=True, stop=True)
            tot_sb = consts.tile([1, BL], f32)
            nc.scalar.activation(tot_sb, tot_ps,
                                 mybir.ActivationFunctionType.Identity,
                                 bias=eps_t, scale=1.0)
            rec = consts.tile([1, BL], f32)
            nc.vector.reciprocal(rec, tot_sb)
            rbc_ps = psum_small_pool.tile([P, BL], f32, tag="small")
            nc.tensor.matmul(rbc_ps, lhsT=ones_row, rhs=rec,
                             start=True, stop=True)
            rbc_sb = consts.tile([P, BL], f32)
            nc.scalar.copy(rbc_sb, rbc_ps)
            an = consts.tile([P, BL, J], f32)
            for b in range(BL):
                nc.scalar.mul(an[:, b, :], au[:, b, :], rbc_sb[:, b:b + 1])
            at_ps = psum_small_pool.tile([BL * J, P], f32, tag="small")
            nc.tensor.transpose(at_ps, an.rearrange("p b j -> p (b j)"),
                                identity32)
            an_t = consts.tile([BL * J, P], f32)
            nc.scalar.copy(an_t, at_ps)
            nc.sync.dma_start(
                out=out_ext.rearrange("b (j p) -> (b j) p", p=P), in_=an_t)

    nc.compile()
    return nc


def _get_nc():
    if "nc" not in _CACHE:
        _CACHE["nc"] = _build()
    return _CACHE["nc"]


def _in_maps(x, y, mask, M):
    x = np.ascontiguousarray(np.asarray(x, dtype=np.float32))
    y = np.ascontiguousarray(np.asarray(y, dtype=np.float32))
    mask = np.ascontiguousarray(np.asarray(mask, dtype=np.int32))
    M = np.ascontiguousarray(np.asarray(M, dtype=np.float32))
    return [
        {
            "x": x[i * BL:(i + 1) * BL],
            "ysl": np.ascontiguousarray(y[:, i * P:(i + 1) * P]),
            "mask": mask[i * BL:(i + 1) * BL],
            "Msl": np.ascontiguousarray(M[:, i * P:(i + 1) * P]),
        }
        for i in range(NCORES)
    ]


def kernel(x, y, mask, M, **_ignored):
    from concourse.bass_utils import run_bass_kernel_spmd

    nc = _get_nc()
    res = run_bass_kernel_spmd(nc, _in_maps(x, y, mask, M),
                               core_ids=list(range(NCORES)))
    out = np.concatenate([res.results[i]["out"] for i in range(NCORES)],
                         axis=0)
    return out.astype(np.float32)


# revision 19
# speedup vs baseline: 1.3402x; 1.3402x over previous
"""Trainium2 Bass kernel for masked attention scoring (sparse_attention).

Computes, per batch b:
    proj = y @ M^T                      # [B, D]
    eij  = tanh(einsum('bsd,bd->bs', x, proj))
    a    = exp(eij) * mask
    a    = a / (sum_s a + EPS)

Sharding: data-parallel over batch B=32 across 8 NeuronCores (4 batches
per core). M is replicated; all reductions stay local per shard.

Per-core device algorithm (memory-bound, x-stream dominated):
  - M rides the sync HWDGE ring as ONE 4 MiB DMA, hard-serialized ahead
    of the x stream (a semaphore dep), so it runs at line rate and proj
    is ready ~27us in; x owns the bandwidth afterwards.
  - proj: M cast to f16 in one VectorE op, transposed on TensorE
    (8 blocks per PSUM bank), proj = yT^T @ M^T accumulated in PSUM
    f32, broadcast across partitions via selector matmuls on TensorE.
  - main pass: stream x in [128, 4, 1024] f32 tiles (natural layout,
    2 MiB DMAs) and compute the d-reduction with ONE fused DVE op per
    s-chunk: scalar_tensor_tensor(mult, mult, accum_out) -> eij column.
  - epilogue: tanh+exp per batch on ScalarE as each batch finishes;
    then mask multiply, free-dim reduce, partition reduce + denominator
    broadcast via tiny TensorE matmuls, normalize, PE-transpose, one
    contiguous DMA out. No strided elementwise DMAs anywhere.
"""

import os
import sys

import numpy as np

for _p in ("/opt/trn_rl_repo",):
    if os.path.isdir(_p) and _p not in sys.path:
        sys.path.insert(0, _p)

B, S, D = 32, 2048, 1024
NCORES = 8
BL = B // NCORES        # batches per core
P = 128                 # SBUF partitions
J = S // P              # 16 s-chunks per batch
HALF = J // 4           # s-chunks per x DMA (2 MiB)
DC = D // P             # 8 d-chunks
EPS = 1e-7

_CACHE = {}


def _build():
    import concourse.bacc as bacc
    import concourse.tile as tile
    from concourse import mybir
    from concourse.masks import make_identity
    from concourse.tile import add_dep_helper

    f32 = mybir.dt.float32
    f16 = mybir.dt.float16
    i32 = mybir.dt.int32

    nc = bacc.Bacc("TRN2", target_bir_lowering=False, debug=False,
                   num_devices=NCORES)

    x_ext = nc.dram_tensor("x", [BL, S, D], f32, kind="ExternalInput").ap()
    y_ext = nc.dram_tensor("y", [BL, D], f32, kind="ExternalInput").ap()
    mask_ext = nc.dram_tensor("mask", [BL, S], i32, kind="ExternalInput").ap()
    m_ext = nc.dram_tensor("M", [D, D], f32, kind="ExternalInput").ap()
    out_ext = nc.dram_tensor("out", [BL, S], f32, kind="ExternalOutput").ap()

    with tile.TileContext(nc) as tc:
        with (
            tc.tile_pool(name="consts", bufs=1) as consts,
            tc.tile_pool(name="psum_t", bufs=2, space="PSUM") as psum_t_pool,
            tc.tile_pool(name="psum_proj", bufs=1, space="PSUM") as psum_proj_pool,
            tc.tile_pool(name="psum_pb", bufs=1, space="PSUM") as psum_pb_pool,
            tc.tile_pool(name="psum_small", bufs=1, space="PSUM") as psum_small_pool,
            tc.tile_pool(name="xpool", bufs=6) as xpool,
            tc.tile_pool(name="scr", bufs=2) as scr_pool,
        ):
            identity16 = consts.tile([P, P], f16)
            make_identity(nc, identity16)
            identity32 = consts.tile([P, P], f32)
            make_identity(nc, identity32)
            ones_col = consts.tile([P, 1], f32)
            nc.vector.memset(ones_col, 1.0)
            ones_row = consts.tile([1, P], f32)
            nc.vector.memset(ones_row, 1.0)
            eps_t = consts.tile([1, 1], f32)
            nc.vector.memset(eps_t, EPS)

            # ---- M^T in f16: ONE 4MB load, one cast, 64 transposes ----
            # mtsb[p_d, ec, dc, e'] = M[ec*128+e', dc*128+p_d]
            mnat = consts.tile([P, DC, D], f32)
            m_dma = nc.sync.dma_start(
                out=mnat,
                in_=m_ext.rearrange("(ec p) d -> p ec d", p=P))
            mnat16 = consts.tile([P, DC, D], f16)
            nc.vector.tensor_copy(mnat16[:, 0:DC // 2, :],
                                  mnat[:, 0:DC // 2, :])
            nc.vector.tensor_copy(mnat16[:, DC // 2:, :],
                                  mnat[:, DC // 2:, :])
            mtsb = consts.tile([P, DC, DC, P], f16)
            for ec in range(DC):
                pt = psum_t_pool.tile([P, DC, P], f16, tag="pt")
                for dc in range(DC):
                    nc.tensor.transpose(pt[:, dc, :],
                                        mnat16[:, ec, dc * P:(dc + 1) * P],
                                        identity16)
                nc.scalar.copy(mtsb[:, ec, :, :], pt)

            # ---- y -> yT in f16 via cast + TensorE transposes ----
            ynat = consts.tile([BL, D], f32)
            nc.sync.dma_start(out=ynat, in_=y_ext)
            ynat16 = consts.tile([BL, D], f16)
            nc.vector.tensor_copy(ynat16, ynat)
            yt_ps = psum_small_pool.tile([P, DC, BL], f16, tag="small")
            for dc in range(DC):
                nc.tensor.transpose(yt_ps[:, dc, :],
                                    ynat16[:, dc * P:(dc + 1) * P],
                                    identity16[:BL, :BL])
            yT = consts.tile([P, DC, BL], f16)
            nc.vector.tensor_copy(yT, yt_ps)

            # ---- proj[b, e] = sum_d y[b, d] * M[e, d]  (PSUM f32) ----
            proj_ps = psum_proj_pool.tile([BL, D], f32)
            for dc in range(DC):
                for eh in range(2):
                    nc.tensor.matmul(
                        proj_ps[:, eh * 512:(eh + 1) * 512],
                        lhsT=yT[:, dc, :],
                        rhs=mtsb[:, eh * 4:(eh + 1) * 4, dc, :],
                        start=(dc == 0),
                        stop=(dc == DC - 1),
                    )
            proj_sb = consts.tile([BL, D], f16)
            nc.vector.tensor_copy(proj_sb, proj_ps)

            # ---- broadcast proj rows across partitions via TensorE ----
            projbc = []
            for b in range(BL):
                sel = consts.tile([BL, P], f16, name=f"sel{b}")
                nc.gpsimd.memset(sel, 0.0)
                nc.gpsimd.affine_select(
                    out=sel, in_=sel,
                    compare_op=mybir.AluOpType.not_equal,
                    fill=1.0, base=-b,
                    pattern=[[0, P]], channel_multiplier=1)
                pb = consts.tile([P, D], f32, name=f"projbc{b}")
                for eh in range(2):
                    pb_ps = psum_pb_pool.tile([P, 512], f32, tag="pbps")
                    nc.tensor.matmul(
                        pb_ps,
                        lhsT=sel,
                        rhs=proj_sb[:, eh * 512:(eh + 1) * 512],
                        start=True, stop=True)
                    if b == 0:
                        nc.vector.tensor_copy(
                            pb[:, eh * 512:(eh + 1) * 512], pb_ps)
                    else:
                        nc.scalar.copy(pb[:, eh * 512:(eh + 1) * 512], pb_ps)
                projbc.append(pb)

            # ---- masks: one contiguous cast-DMA + PE transposes ----
            mk_nat = consts.tile([J, BL, P], f32)
            nc.gpsimd.dma_start(
                out=mk_nat,
                in_=mask_ext.rearrange("b (j p) -> j b p", p=P))
            mask_all = consts.tile([P, BL, J], f32)
            for b in range(BL):
                mk_ps = psum_small_pool.tile([P, J], f32, tag="small")
                nc.tensor.transpose(mk_ps, mk_nat[:, b, :], identity32[:J, :J])
                nc.scalar.copy(mask_all[:, b, :], mk_ps)

            # ---- main pass: eij[p, b, col] = x[b, s, :] . proj[b, :] ----
            eij = consts.tile([P, BL, J], f32)
            th = consts.tile([P, BL, J], f32)
            ex = consts.tile([P, BL, J], f32)
            first_x_dma = None
            for b in range(BL):
                for half in range(4):
                    xt = xpool.tile([P, HALF, D], f32, tag="xt")
                    xd = nc.sync.dma_start(
                        out=xt,
                        in_=x_ext[b, half * HALF * P:(half + 1) * HALF * P, :]
                        .rearrange("(j p) d -> p j d", p=P),
                    )
                    if first_x_dma is None:
                        first_x_dma = xd
                    for j in range(HALF):
                        col = half * HALF + j
                        scr = scr_pool.tile([P, D], f32, tag="scr")
                        nc.vector.scalar_tensor_tensor(
                            out=scr,
                            in0=xt[:, j, :],
                            scalar=1.0,
                            in1=projbc[b],
                            op0=mybir.AluOpType.mult,
                            op1=mybir.AluOpType.mult,
                            accum_out=eij[:, b, col:col + 1],
                        )
                nc.scalar.activation(th[:, b, :], eij[:, b, :],
                                     mybir.ActivationFunctionType.Tanh)
                nc.scalar.activation(ex[:, b, :], th[:, b, :],
                                     mybir.ActivationFunctionType.Exp)

            # x stream starts only after the M load has drained
            add_dep_helper(first_x_dma.ins, m_dma.ins, sync=True,
                           reason="x stream after M load")

            # ---- fused epilogue over all batches ----
            au = consts.tile([P, BL, J], f32)
            nc.vector.tensor_mul(au, ex, mask_all)
            cs = consts.tile([P, BL], f32)
            nc.vector.reduce_sum(cs, au, axis=mybir.AxisListType.X)
            tot_ps = psum_small_pool.tile([1, BL], f32, tag="small")
            nc.tensor.matmul(tot_ps, lhsT=ones_col, rhs=cs,
                             start=True, stop=True)
            tot_sb = consts.tile([1, BL], f32)
            nc.scalar.activation(tot_sb, tot_ps,
                                 mybir.ActivationFunctionType.Identity,
                                 bias=eps_t, scale=1.0)
            rec = consts.tile([1, BL], f32)
            nc.vector.reciprocal(rec, tot_sb)
            rbc_ps = psum_small_pool.tile([P, BL], f32, tag="small")
            nc.tensor.matmul(rbc_ps, lhsT=ones_row, rhs=rec,
                             start=True, stop=True)
            rbc_sb = consts.tile([P, BL], f32)
            nc.scalar.copy(rbc_sb, rbc_ps)
            an = consts.tile([P, BL, J], f32)
            for b in range(BL):
                nc.scalar.mul(an[:, b, :], au[:, b, :], rbc_sb[:, b:b + 1])
            at_ps = psum_small_pool.tile([BL * J, P], f32, tag="small")
            nc.tensor.transpose(at_ps, an.rearrange("p b j -> p (b j)"),
                                identity32)
            an_t = consts.tile([BL * J, P], f32)
            nc.scalar.copy(an_t, at_ps)
            nc.sync.dma_start(
                out=out_ext.rearrange("b (j p) -> (b j) p", p=P), in_=an_t)

    nc.compile()
    return nc


def _get_nc():
    if "nc" not in _CACHE:
        _CACHE["nc"] = _build()
    return _CACHE["nc"]


def _in_maps(x, y, mask, M):
    x = np.ascontiguousarray(np.asarray(x, dtype=np.float32))
    y = np.ascontiguousarray(np.asarray(y, dtype=np.float32))
    mask = np.ascontiguousarray(np.asarray(mask, dtype=np.int32))
    M = np.ascontiguousarray(np.asarray(M, dtype=np.float32))
    return [
        {
            "x": x[i * BL:(i + 1) * BL],
            "y": y[i * BL:(i + 1) * BL],
            "mask": mask[i * BL:(i + 1) * BL],
            "M": M,
        }
        for i in range(NCORES)
    ]


def kernel(x, y, mask, M, **_ignored):
    from concourse.bass_utils import run_bass_kernel_spmd

    nc = _get_nc()
    res = run_bass_kernel_spmd(nc, _in_maps(x, y, mask, M),
                               core_ids=list(range(NCORES)))
    out = np.concatenate([res.results[i]["out"] for i in range(NCORES)],
                         axis=0)
    return out.astype(np.float32)


# revision 20
# speedup vs baseline: 1.4611x; 1.0902x over previous
"""Trainium2 Bass kernel for masked attention scoring (sparse_attention).

Computes, per batch b:
    proj = y @ M^T                      # [B, D]
    eij  = tanh(einsum('bsd,bd->bs', x, proj))
    a    = exp(eij) * mask
    a    = a / (sum_s a + EPS)

Sharding: data-parallel over batch B=32 across 8 NeuronCores (4 batches
per core). M is replicated; all reductions stay local per shard.

Per-core device algorithm (memory-bound, x-stream dominated):
  - M rides the sync HWDGE ring as ONE 4 MiB DMA, hard-serialized ahead
    of the x stream (a semaphore dep), so it runs at line rate and proj
    is ready ~27us in; x owns the bandwidth afterwards.
  - proj: M cast to f16 in one VectorE op, transposed on TensorE
    (8 blocks per PSUM bank), proj = yT^T @ M^T accumulated in PSUM
    f32, broadcast across partitions via selector matmuls on TensorE.
  - main pass: stream x in [128, 4, 1024] f32 tiles (natural layout,
    2 MiB DMAs) and compute the d-reduction with ONE fused DVE op per
    s-chunk: scalar_tensor_tensor(mult, mult, accum_out) -> eij column.
  - epilogue: tanh+exp per batch on ScalarE as each batch finishes;
    then mask multiply, free-dim reduce, partition reduce + denominator
    broadcast via tiny TensorE matmuls, normalize, PE-transpose, one
    contiguous DMA out. No strided elementwise DMAs anywhere.
"""

import os
import sys

import numpy as np

for _p in ("/opt/trn_rl_repo",):
    if os.path.isdir(_p) and _p not in sys.path:
        sys.path.insert(0, _p)

B, S, D = 32, 2048, 1024
NCORES = 8
BL = B // NCORES        # batches per core
P = 128                 # SBUF partitions
J = S // P              # 16 s-chunks per batch
HALF = J // 4           # s-chunks per x DMA (2 MiB)
DC = D // P             # 8 d-chunks
EPS = 1e-7

_CACHE = {}


def _build():
    import concourse.bacc as bacc
    import concourse.tile as tile
    from concourse import mybir
    from concourse.masks import make_identity
    from concourse.tile import add_dep_helper

    f32 = mybir.dt.float32
    f16 = mybir.dt.float16
    i32 = mybir.dt.int32

    nc = bacc.Bacc("TRN2", target_bir_lowering=False, debug=False,
                   num_devices=NCORES)

    x_ext = nc.dram_tensor("x", [BL, S, D], f32, kind="ExternalInput").ap()
    y_ext = nc.dram_tensor("y16", [BL, D], f16, kind="ExternalInput").ap()
    mask_ext = nc.dram_tensor("mask", [BL, S], i32, kind="ExternalInput").ap()
    m_ext = nc.dram_tensor("M16", [D, D], f16, kind="ExternalInput").ap()
    out_ext = nc.dram_tensor("out", [BL, S], f32, kind="ExternalOutput").ap()

    with tile.TileContext(nc) as tc:
        with (
            tc.tile_pool(name="consts", bufs=1) as consts,
            tc.tile_pool(name="psum_t", bufs=2, space="PSUM") as psum_t_pool,
            tc.tile_pool(name="psum_proj", bufs=1, space="PSUM") as psum_proj_pool,
            tc.tile_pool(name="psum_pb", bufs=1, space="PSUM") as psum_pb_pool,
            tc.tile_pool(name="psum_small", bufs=1, space="PSUM") as psum_small_pool,
            tc.tile_pool(name="xpool", bufs=6) as xpool,
            tc.tile_pool(name="scr", bufs=2) as scr_pool,
        ):
            identity16 = consts.tile([P, P], f16)
            make_identity(nc, identity16)
            identity32 = consts.tile([P, P], f32)
            make_identity(nc, identity32)
            ones_col = consts.tile([P, 1], f32)
            nc.vector.memset(ones_col, 1.0)
            ones_row = consts.tile([1, P], f32)
            nc.vector.memset(ones_row, 1.0)
            eps_t = consts.tile([1, 1], f32)
            nc.vector.memset(eps_t, EPS)

            # ---- M^T in f16 via hardware xbar transpose DMAs ----
            # mtsb[p_dd, dc, e] = M[e, dc*128+p_dd]
            mtsb = consts.tile([P, DC, D], f16)
            m_dmas = []
            for dc in range(DC):
                m_dmas.append(nc.sync.dma_start_transpose(
                    mtsb[:, dc, :], m_ext[:, dc * P:(dc + 1) * P]))

            # ---- y -> yT in f16 via TensorE transposes ----
            ynat16 = consts.tile([BL, D], f16)
            nc.sync.dma_start(out=ynat16, in_=y_ext)
            yt_ps = psum_small_pool.tile([P, DC, BL], f16, tag="small")
            for dc in range(DC):
                nc.tensor.transpose(yt_ps[:, dc, :],
                                    ynat16[:, dc * P:(dc + 1) * P],
                                    identity16[:BL, :BL])
            yT = consts.tile([P, DC, BL], f16)
            nc.vector.tensor_copy(yT, yt_ps)

            # ---- proj[b, e] = sum_d y[b, d] * M[e, d]  (PSUM f32) ----
            proj_ps = psum_proj_pool.tile([BL, D], f32)
            for dc in range(DC):
                for eh in range(2):
                    nc.tensor.matmul(
                        proj_ps[:, eh * 512:(eh + 1) * 512],
                        lhsT=yT[:, dc, :],
                        rhs=mtsb[:, dc, eh * 512:(eh + 1) * 512],
                        start=(dc == 0),
                        stop=(dc == DC - 1),
                    )
            proj_sb = consts.tile([BL, D], f16)
            nc.vector.tensor_copy(proj_sb, proj_ps)

            # ---- broadcast proj rows across partitions via TensorE ----
            projbc = []
            for b in range(BL):
                sel = consts.tile([BL, P], f16, name=f"sel{b}")
                nc.gpsimd.memset(sel, 0.0)
                nc.gpsimd.affine_select(
                    out=sel, in_=sel,
                    compare_op=mybir.AluOpType.not_equal,
                    fill=1.0, base=-b,
                    pattern=[[0, P]], channel_multiplier=1)
                pb = consts.tile([P, D], f32, name=f"projbc{b}")
                for eh in range(2):
                    pb_ps = psum_pb_pool.tile([P, 512], f32, tag="pbps")
                    nc.tensor.matmul(
                        pb_ps,
                        lhsT=sel,
                        rhs=proj_sb[:, eh * 512:(eh + 1) * 512],
                        start=True, stop=True)
                    if b == 0:
                        nc.vector.tensor_copy(
                            pb[:, eh * 512:(eh + 1) * 512], pb_ps)
                    else:
                        nc.scalar.copy(pb[:, eh * 512:(eh + 1) * 512], pb_ps)
                projbc.append(pb)

            # ---- masks: one contiguous cast-DMA + PE transposes ----
            mk_nat = consts.tile([J, BL, P], f32)
            nc.gpsimd.dma_start(
                out=mk_nat,
                in_=mask_ext.rearrange("b (j p) -> j b p", p=P))
            mask_all = consts.tile([P, BL, J], f32)
            for b in range(BL):
                mk_ps = psum_small_pool.tile([P, J], f32, tag="small")
                nc.tensor.transpose(mk_ps, mk_nat[:, b, :], identity32[:J, :J])
                nc.scalar.copy(mask_all[:, b, :], mk_ps)

            # ---- main pass: eij[p, b, col] = x[b, s, :] . proj[b, :] ----
            eij = consts.tile([P, BL, J], f32)
            th = consts.tile([P, BL, J], f32)
            ex = consts.tile([P, BL, J], f32)
            first_x_dma = None
            for b in range(BL):
                for half in range(4):
                    xt = xpool.tile([P, HALF, D], f32, tag="xt")
                    xd = nc.sync.dma_start(
                        out=xt,
                        in_=x_ext[b, half * HALF * P:(half + 1) * HALF * P, :]
                        .rearrange("(j p) d -> p j d", p=P),
                    )
                    if first_x_dma is None:
                        first_x_dma = xd
                    for j in range(HALF):
                        col = half * HALF + j
                        scr = scr_pool.tile([P, D], f32, tag="scr")
                        nc.vector.scalar_tensor_tensor(
                            out=scr,
                            in0=xt[:, j, :],
                            scalar=1.0,
                            in1=projbc[b],
                            op0=mybir.AluOpType.mult,
                            op1=mybir.AluOpType.mult,
                            accum_out=eij[:, b, col:col + 1],
                        )
                nc.scalar.activation(th[:, b, :], eij[:, b, :],
                                     mybir.ActivationFunctionType.Tanh)
                nc.scalar.activation(ex[:, b, :], th[:, b, :],
                                     mybir.ActivationFunctionType.Exp)

            # x stream starts only after the M^T transpose DMAs drained
            for md in m_dmas:
                add_dep_helper(first_x_dma.ins, md.ins, sync=True,
                               reason="x stream after M transpose loads")

            # ---- fused epilogue over all batches ----
            au = consts.tile([P, BL, J], f32)
            nc.vector.tensor_mul(au, ex, mask_all)
            cs = consts.tile([P, BL], f32)
            nc.vector.reduce_sum(cs, au, axis=mybir.AxisListType.X)
            tot_ps = psum_small_pool.tile([1, BL], f32, tag="small")
            nc.tensor.matmul(tot_ps, lhsT=ones_col, rhs=cs,
                             start=True, stop=True)
            tot_sb = consts.tile([1, BL], f32)
            nc.scalar.activation(tot_sb, tot_ps,
                                 mybir.ActivationFunctionType.Identity,
                                 bias=eps_t, scale=1.0)
            rec = consts.tile([1, BL], f32)
            nc.vector.reciprocal(rec, tot_sb)
            rbc_ps = psum_small_pool.tile([P, BL], f32, tag="small")
            nc.tensor.matmul(rbc_ps, lhsT=ones_row, rhs=rec,
                             start=True, stop=True)
            rbc_sb = consts.tile([P, BL], f32)
            nc.scalar.copy(rbc_sb, rbc_ps)
            an = consts.tile([P, BL, J], f32)
            for b in range(BL):
                nc.scalar.mul(an[:, b, :], au[:, b, :], rbc_sb[:, b:b + 1])
            at_ps = psum_small_pool.tile([BL * J, P], f32, tag="small")
            nc.tensor.transpose(at_ps, an.rearrange("p b j -> p (b j)"),
                                identity32)
            an_t = consts.tile([BL * J, P], f32)
            nc.scalar.copy(an_t, at_ps)
            nc.sync.dma_start(
                out=out_ext.rearrange("b (j p) -> (b j) p", p=P), in_=an_t)

    nc.compile()
    return nc


def _get_nc():
    if "nc" not in _CACHE:
        _CACHE["nc"] = _build()
    return _CACHE["nc"]


def _in_maps(x, y, mask, M):
    x = np.ascontiguousarray(np.asarray(x, dtype=np.float32))
    y16 = np.ascontiguousarray(np.asarray(y, dtype=np.float32)
                               .astype(np.float16))
    mask = np.ascontiguousarray(np.asarray(mask, dtype=np.int32))
    M16 = np.ascontiguousarray(np.asarray(M, dtype=np.float32)
                               .astype(np.float16))
    return [
        {
            "x": x[i * BL:(i + 1) * BL],
            "y16": y16[i * BL:(i + 1) * BL],
            "mask": mask[i * BL:(i + 1) * BL],
            "M16": M16,
        }
        for i in range(NCORES)
    ]


def kernel(x, y, mask, M, **_ignored):
    from concourse.bass_utils import run_bass_kernel_spmd

    nc = _get_nc()
    res = run_bass_kernel_spmd(nc, _in_maps(x, y, mask, M),
                               core_ids=list(range(NCORES)))
    out = np.concatenate([res.results[i]["out"] for i in range(NCORES)],
                         axis=0)
    return out.astype(np.float32)
